# revision 4
# baseline (speedup 1.0000x reference)
"""Causal self-attention Trainium2 kernel (8-core head-parallel tensor parallel).

v3 strategy (per core, 2 heads, feature-major dataflow):
  - Phase 1: b0's QKV (x^T k-tile-split DMAs, f32r matmuls, DVE bias
    eviction) + b0's V transpose into token-major v_sb.
  - Attention per (b, q-chunk) group, head-major, with the causal mask
    applied POST-exp by GpSimd affine_select (zero the future triangle),
    Z accumulated for free via a ones column in V, and the normalize
    chain: DVE copy Z to SBUF -> reciprocal_approx_fast (PSUM source is
    broken on HW) -> GpSimd partition_broadcast -> DVE multiply.
  - The PE is kept continuously busy (HAM clock-gate stays at 2.4 GHz)
    by interleaving filler matmuls into every attention k-tile step:
    b1's QKV + V-transpose matmuls fill b0's attention; projection
    matmuls of completed groups fill b1's attention.
  - Host: sum 8 bf16 partial out^T in f32, transpose, +b_proj.
"""

import sys

if "/opt/trn_rl_repo" not in sys.path:
    sys.path.insert(0, "/opt/trn_rl_repo")

import numpy as np

# ---- problem constants (hardcoded for the grading harness) ----
B, T, C, H = 2, 2048, 1024, 16
HD = C // H            # 64
N_CORES = 8
HPC = H // N_CORES     # heads per core = 2

_F32R = True


def _cfg_full():
    return dict(B=B, T=T, C=C, HPC=HPC, f32r=_F32R)


def build_nc(cfg):
    """Build the single-core SPMD Bass program."""
    import concourse.bacc as bacc
    import concourse.mybir as mybir
    import concourse.tile as tile
    from concourse.masks import make_identity

    Bc, Tc, Cc, hpc = cfg["B"], cfg["T"], cfg["C"], cfg["HPC"]
    f32r = mybir.dt.float32r if cfg["f32r"] else mybir.dt.float32
    f32 = mybir.dt.float32
    bf16 = mybir.dt.bfloat16
    BT = Bc * Tc
    MQ = hpc * HD                 # 128
    assert MQ == 128
    KT_C = Cc // 128              # 8
    TOKC = 512
    NCH = BT // TOKC              # 8
    QC = Tc // TOKC               # 4
    KTT = Tc // 128               # 16
    MO = Cc // 128                # 8
    CH_PER_B = Tc // TOKC         # 4
    DKT = TOKC // 128             # 4

    nc = bacc.Bacc()
    xT = nc.declare_dram_parameter("xT", [Cc, BT], f32r, isOutput=False)
    wqkvT = nc.declare_dram_parameter("wqkvT", [Cc, 3 * MQ], f32r, isOutput=False)
    bqkv = nc.declare_dram_parameter("bqkv", [3 * MQ, 1], f32, isOutput=False)
    wpT = nc.declare_dram_parameter("wpT", [MQ, Cc], bf16, isOutput=False)
    outT = nc.declare_dram_parameter("outT", [Cc, BT], bf16, isOutput=True)

    xT_r = xT.rearrange("(kt p) t -> p kt t", p=128)
    wq_r = wqkvT.rearrange("(kt p) m -> p kt m", p=128)
    bq_r = bqkv.rearrange("(g p) o -> p (g o)", p=128)

    AF = mybir.ActivationFunctionType

    with tile.TileContext(nc) as tc:
        with (
            tc.tile_pool(name="consts", bufs=1) as consts,
            tc.tile_pool(name="xpool", bufs=12) as xpool,
            tc.tile_pool(name="epool", bufs=4) as epool,
            tc.tile_pool(name="npool", bufs=2) as npool,
            tc.tile_pool(name="ypool", bufs=2) as ypool,
            tc.tile_pool(name="opool", bufs=4) as opool,
            tc.tile_pool(name="ps_mm", bufs=2, space="PSUM") as ps_mm,
            tc.tile_pool(name="ps_s", bufs=2, space="PSUM") as ps_s,
            tc.tile_pool(name="ps_y", bufs=2, space="PSUM") as ps_y,
            tc.tile_pool(name="ps_o", bufs=2, space="PSUM") as ps_o,
        ):
            # ---- const DMAs ----
            w_t = []
            for kt in range(KT_C):
                w = consts.tile([128, 3 * MQ], f32r, tag=f"w{kt}", name=f"w{kt}")
                nc.sync.dma_start(out=w, in_=wq_r[:, kt, :])
                w_t.append(w)
            b_sb = consts.tile([128, 3], f32, tag="b")
            nc.sync.dma_start(out=b_sb, in_=bq_r)
            wp_sb = consts.tile([128, Cc], bf16, tag="wp")
            nc.sync.dma_start(out=wp_sb, in_=wpT[:, :])

            qT_sb = consts.tile([128, BT], f32r, tag="qT")
            kT_sb = consts.tile([128, BT], f32r, tag="kT")
            vT_sb = consts.tile([128, BT], bf16, tag="vT")

            ident = consts.tile([128, 128], f32, tag="ident")
            make_identity(nc, ident)
            ident_bf = consts.tile([128, 128], bf16, tag="ident_bf")
            nc.vector.tensor_copy(ident_bf[:, :], ident[:, :])

            v_sb = [
                consts.tile([128, KTT, hpc, 65], bf16, tag=f"v{b}",
                            name=f"v{b}") for b in range(Bc)
            ]
            for b in range(Bc):
                nc.vector.memset(v_sb[b][:, :, :, 64:65], 1.0)

            # all x slice DMAs upfront; the in-order sync DMA queue plus
            # the 12-buffer pool gives a self-throttling ~1.5-chunk
            # prefetch ahead of consumption
            x_tiles = {}
            for ch in range(NCH):
                for kt in range(KT_C):
                    x = xpool.tile([128, TOKC], f32r, tag="x",
                                   name=f"x{ch}_{kt}")
                    nc.sync.dma_start(
                        out=x, in_=xT_r[:, kt, ch * TOKC:(ch + 1) * TOKC])
                    x_tiles[(ch, kt)] = x

            # ---- filler machinery: one PE-sized thunk per pop ----
            fillers = []          # list of (ready_iter, thunk) in order
            iter_ctr = [0]

            def push(thunk, delay=0):
                fillers.append((iter_ctr[0] + delay, thunk))

            def pop_fillers(budget):
                n = 0
                while fillers and n < budget and fillers[0][0] <= iter_ctr[0]:
                    fillers.pop(0)[1]()
                    n += 1

            def flush_fillers():
                while fillers:
                    fillers.pop(0)[1]()

            # ---- QKV chunk (emitted directly in phase 1, or pushed) ----
            def qkv_mm_group(ch, m):
                ps = ps_mm.tile([128, TOKC], f32, tag="mm")
                for kt in range(KT_C):
                    nc.tensor.matmul(
                        ps[:, :],
                        w_t[kt][:, m * MQ:(m + 1) * MQ],
                        x_tiles[(ch, kt)][:, :],
                        start=(kt == 0), stop=(kt == KT_C - 1),
                    )
                dst = (qT_sb, kT_sb, vT_sb)[m]
                nc.vector.tensor_scalar_add(
                    out=dst[:, ch * TOKC:(ch + 1) * TOKC], in0=ps[:, :],
                    scalar1=b_sb[:, m:m + 1],
                )

            def push_qkv(ch):
                for m in range(3):
                    push(lambda ch=ch, m=m: qkv_mm_group(ch, m))

            def vtrans_tile(b, kt, on_act):
                ps_t = ps_mm.tile([128, 128], bf16, tag="mm")
                nc.tensor.transpose(
                    ps_t[:, :],
                    vT_sb[:, b * Tc + kt * 128:b * Tc + (kt + 1) * 128],
                    ident_bf[:, :],
                )
                for hh in range(hpc):
                    if on_act:
                        nc.scalar.activation(
                            out=v_sb[b][:, kt, hh, 0:64],
                            in_=ps_t[:, hh * HD:(hh + 1) * HD], func=AF.Copy)
                    else:
                        nc.vector.tensor_copy(
                            v_sb[b][:, kt, hh, 0:64],
                            ps_t[:, hh * HD:(hh + 1) * HD])

            def proj_mo(b, qc, yT_t, mo):
                q_sl = slice(b * Tc + qc * TOKC, b * Tc + (qc + 1) * TOKC)
                pso = ps_o.tile([128, TOKC], f32, tag="o")
                nc.tensor.matmul(
                    pso[:, :],
                    wp_sb[:, mo * 128:(mo + 1) * 128],
                    yT_t[:, :],
                    start=True, stop=True,
                )
                o_t = opool.tile([128, TOKC], bf16, tag="o")
                if mo % 2 == 0:
                    nc.vector.tensor_copy(o_t[:, :], pso[:, :])
                else:
                    nc.scalar.activation(out=o_t[:, :], in_=pso[:, :],
                                         func=AF.Copy)
                nc.gpsimd.dma_start(
                    out=outT[mo * 128:(mo + 1) * 128, q_sl], in_=o_t[:, :])

            def push_proj(b, qc, yT_t, delay=3):
                for mo in range(MO):
                    push(lambda mo=mo: proj_mo(b, qc, yT_t, mo), delay=delay)

            DEPTH = 2

            def emit_group(b, qc):
                n_kt = (qc + 1) * DKT
                if qc == 0:
                    kts = list(range(DKT))
                else:
                    nd = list(range(0, qc * DKT))
                    dg = list(range(qc * DKT, qc * DKT + DKT))
                    kts = nd[:2] + dg + nd[2:]
                q_sl = slice(b * Tc + qc * TOKC, b * Tc + (qc + 1) * TOKC)
                yT_t = ypool.tile([128, TOKC], bf16, tag="yT")
                for hh in range(hpc):
                    psy = ps_y.tile([65, TOKC], f32, tag="y")
                    pend = []
                    n_av = 0

                    def emit_av(kt, e_t):
                        nonlocal n_av
                        nc.tensor.matmul(
                            psy[:, :],
                            v_sb[b][:, kt, hh, :],
                            e_t[:, :],
                            start=(n_av == 0), stop=(n_av == n_kt - 1),
                        )
                        n_av += 1

                    for kt in kts:
                        pss = ps_s.tile([128, TOKC], f32, tag="s")
                        nc.tensor.matmul(
                            pss[:, :],
                            kT_sb[hh * HD:(hh + 1) * HD,
                                  b * Tc + kt * 128:b * Tc + (kt + 1) * 128],
                            qT_sb[hh * HD:(hh + 1) * HD, q_sl],
                            start=True, stop=True,
                        )
                        e_t = epool.tile([128, TOKC], bf16, tag="e")
                        nc.scalar.activation(out=e_t[:, :], in_=pss[:, :],
                                             func=AF.Exp, scale=0.125)
                        di = kt - qc * DKT
                        if di >= 0:
                            nc.gpsimd.affine_select(
                                out=e_t[:, :], in_=e_t[:, :],
                                compare_op=mybir.AluOpType.is_ge,
                                fill=0.0,
                                base=-(di * 128),
                                pattern=[[1, TOKC]],
                                channel_multiplier=-1,
                            )
                        pend.append((kt, e_t))
                        pop_fillers(3)
                        iter_ctr[0] += 1
                        if len(pend) > DEPTH:
                            emit_av(*pend.pop(0))
                    for kt, e_t in pend:
                        emit_av(kt, e_t)

                    # normalize (Z must bounce through SBUF: the custom
                    # DVE reciprocal misreads PSUM sources on HW)
                    zrow = npool.tile([1, TOKC], f32, tag="z")
                    nc.vector.tensor_copy(zrow[:, :], psy[64:65, :])
                    rc = npool.tile([1, TOKC], f32, tag="rc")
                    nc.vector.reciprocal_approx_fast(rc[:, :], zrow[:, :])
                    rcb = npool.tile([64, TOKC], f32, tag="rcb")
                    nc.gpsimd.partition_broadcast(rcb[:, :], rc[:, :])
                    nc.vector.tensor_mul(
                        yT_t[hh * HD:(hh + 1) * HD, :],
                        psy[0:HD, :], rcb[:, :],
                    )
                return yT_t

            # ---- schedule ----
            for ch in range(CH_PER_B):               # b0 QKV, direct
                for m in range(3):
                    qkv_mm_group(ch, m)
            for kt in range(KTT):                    # b0 V transpose
                vtrans_tile(0, kt, on_act=True)

            # b1 work rides as filler inside b0's attention
            for ch in range(CH_PER_B, NCH):
                push_qkv(ch)
            for kt in range(KTT):
                push(lambda kt=kt: vtrans_tile(1, kt, on_act=False))

            for qc in range(QC):
                yT_t = emit_group(0, qc)
                push_proj(0, qc, yT_t)
            flush_fillers()                          # b1 QKV/vtrans must
                                                     # precede b1 groups
            for qc in range(QC):
                yT_t = emit_group(1, qc)
                push_proj(1, qc, yT_t)
            flush_fillers()

    nc.finalize()
    return nc


def prep_inputs(cfg, x, W_attn, b_attn, W_proj, b_proj):
    """Host-side sharding: returns per-core input dicts."""
    Bc, Tc, Cc, hpc = cfg["B"], cfg["T"], cfg["C"], cfg["HPC"]
    n_cores = (Cc // HD) // hpc
    BT = Bc * Tc
    MQ = hpc * HD

    x = np.ascontiguousarray(x, dtype=np.float32)
    xT = np.ascontiguousarray(x.reshape(BT, Cc).T)

    in_maps = []
    for c in range(n_cores):
        r0 = c * MQ
        rows = []
        for g in range(3):
            rows.append(np.arange(g * Cc + r0, g * Cc + r0 + MQ))
        rows = np.concatenate(rows)
        w_slice = W_attn[rows, :]                       # [384, C]
        wqkvT = np.ascontiguousarray(w_slice.T)         # [C, 384]
        bq = np.ascontiguousarray(b_attn[rows].reshape(MQ * 3, 1))
        import ml_dtypes
        wpT = np.ascontiguousarray(W_proj[:, r0:r0 + MQ].T).astype(ml_dtypes.bfloat16)
        in_maps.append({
            "xT": xT,
            "wqkvT": wqkvT.astype(np.float32),
            "bqkv": bq.astype(np.float32),
            "wpT": wpT,
        })
    return in_maps


def combine(cfg, results, b_proj):
    Bc, Tc, Cc = cfg["B"], cfg["T"], cfg["C"]
    acc = results[0]["outT"].astype(np.float32)
    for r in results[1:]:
        acc = acc + r["outT"].astype(np.float32)
    out = acc.T + b_proj[None, :]
    return np.ascontiguousarray(out.reshape(Bc, Tc, Cc).astype(np.float32))


_NC_CACHE = {}


def kernel(x, W_attn, b_attn, W_proj, b_proj):
    from concourse.bass_utils import run_bass_kernel_spmd

    cfg = _cfg_full()
    key = "full"
    if key not in _NC_CACHE:
        _NC_CACHE[key] = build_nc(cfg)
    nc = _NC_CACHE[key]
    in_maps = prep_inputs(cfg, np.asarray(x), np.asarray(W_attn),
                          np.asarray(b_attn), np.asarray(W_proj),
                          np.asarray(b_proj))
    res = run_bass_kernel_spmd(nc, in_maps, list(range(N_CORES)))
    return combine(cfg, res.results, np.asarray(b_proj, dtype=np.float32))


# revision 8
# speedup vs baseline: 1.5211x; 1.5211x over previous
"""Causal self-attention Trainium2 kernel (8-core head-parallel tensor parallel).

v4 strategy (per core, 2 heads, feature-major dataflow):
  - QKV: qkv^T = W^T.T @ x^T per 512-token chunk (f32r matmuls, DVE
    bias eviction via tensor_scalar_add). Chunk 0 runs up front; all
    later chunks ride as PE filler inside the attention stream.
  - Attention per (b, q-chunk) group, k-tile loop with BOTH heads per
    step: the two S matmuls use disjoint PE row groups (partitions 0-63
    / 64-127) so they execute concurrently; their outputs land in one
    2-bank PSUM pair tile, giving ONE [128,1024] exp on ACT and ONE
    causal-mask affine_select on GpSimd per diagonal k-tile.
  - Z rides as a ones-column in V (AV row 64). Normalize: DVE copies Z
    to SBUF (custom DVE reciprocal misreads PSUM on HW), DVE
    reciprocal_approx_fast, GpSimd partition_broadcast, DVE multiply.
  - PE never idles: a filler queue (later QKV chunks, V transposes) and
    a proj queue (projection matmuls of completed groups) are popped
    every k-tile step, keeping the HAM clock-gate at 2.4 GHz.
  - Host: sum 8 bf16 partial out^T in f32, transpose, +b_proj.
"""

import sys

if "/opt/trn_rl_repo" not in sys.path:
    sys.path.insert(0, "/opt/trn_rl_repo")

import numpy as np

# ---- problem constants (hardcoded for the grading harness) ----
B, T, C, H = 2, 2048, 1024, 16
HD = C // H            # 64
N_CORES = 8
HPC = H // N_CORES     # heads per core = 2

_F32R = True


def _cfg_full():
    return dict(B=B, T=T, C=C, HPC=HPC, f32r=_F32R)


def build_nc(cfg):
    """Build the single-core SPMD Bass program."""
    import concourse.bacc as bacc
    import concourse.mybir as mybir
    import concourse.tile as tile
    from concourse.masks import make_identity

    Bc, Tc, Cc, hpc = cfg["B"], cfg["T"], cfg["C"], cfg["HPC"]
    f32r = mybir.dt.float32r if cfg["f32r"] else mybir.dt.float32
    f32 = mybir.dt.float32
    bf16 = mybir.dt.bfloat16
    BT = Bc * Tc
    MQ = hpc * HD                 # 128
    assert MQ == 128
    KT_C = Cc // 128              # 8
    TOKC = 512
    NCH = BT // TOKC              # 8
    QC = Tc // TOKC               # 4
    KTT = Tc // 128               # 16
    MO = Cc // 128                # 8
    CH_PER_B = Tc // TOKC         # 4
    DKT = TOKC // 128             # 4

    nc = bacc.Bacc()
    xT = nc.declare_dram_parameter("xT", [Cc, BT], f32r, isOutput=False)
    wqkvT = nc.declare_dram_parameter("wqkvT", [Cc, 3 * MQ], f32r, isOutput=False)
    bqkv = nc.declare_dram_parameter("bqkv", [3 * MQ, 1], f32, isOutput=False)
    wpT = nc.declare_dram_parameter("wpT", [MQ, Cc], bf16, isOutput=False)
    outT = nc.declare_dram_parameter("outT", [Cc, BT], bf16, isOutput=True)

    xT_r = xT.rearrange("(kt p) t -> p kt t", p=128)
    wq_r = wqkvT.rearrange("(kt p) m -> p kt m", p=128)
    bq_r = bqkv.rearrange("(g p) o -> p (g o)", p=128)

    AF = mybir.ActivationFunctionType

    with tile.TileContext(nc) as tc:
        with (
            tc.tile_pool(name="consts", bufs=1) as consts,
            tc.tile_pool(name="xpool", bufs=12) as xpool,
            tc.tile_pool(name="epool", bufs=4) as epool,
            tc.tile_pool(name="npool", bufs=2) as npool,
            tc.tile_pool(name="ypool", bufs=2) as ypool,
            tc.tile_pool(name="opool", bufs=4) as opool,
            tc.tile_pool(name="ps_x", bufs=2, space="PSUM") as ps_x,
            tc.tile_pool(name="ps_s", bufs=2, space="PSUM") as ps_s,
            tc.tile_pool(name="ps_y", bufs=2, space="PSUM") as ps_y,
        ):
            # ---- const DMAs ----
            w_t = []
            for kt in range(KT_C):
                w = consts.tile([128, 3 * MQ], f32r, tag=f"w{kt}", name=f"w{kt}")
                nc.sync.dma_start(out=w, in_=wq_r[:, kt, :])
                w_t.append(w)
            b_sb = consts.tile([128, 3], f32, tag="b")
            nc.sync.dma_start(out=b_sb, in_=bq_r)
            wp_sb = consts.tile([128, Cc], bf16, tag="wp")
            nc.sync.dma_start(out=wp_sb, in_=wpT[:, :])

            qT_sb = consts.tile([128, BT], f32r, tag="qT")
            kT_sb = consts.tile([128, BT], f32r, tag="kT")
            vT_sb = consts.tile([128, BT], bf16, tag="vT")

            ident = consts.tile([128, 128], f32, tag="ident")
            make_identity(nc, ident)
            ident_bf = consts.tile([128, 128], bf16, tag="ident_bf")
            nc.vector.tensor_copy(ident_bf[:, :], ident[:, :])

            v_sb = [
                consts.tile([128, KTT, hpc, 65], bf16, tag=f"v{b}",
                            name=f"v{b}") for b in range(Bc)
            ]
            for b in range(Bc):
                nc.vector.memset(v_sb[b][:, :, :, 64:65], 1.0)

            # all x slice DMAs upfront (sync queue + 12-buffer pool =
            # self-throttling prefetch)
            x_tiles = {}
            for ch in range(NCH):
                for kt in range(KT_C):
                    x = xpool.tile([128, TOKC], f32r, tag="x",
                                   name=f"x{ch}_{kt}")
                    nc.sync.dma_start(
                        out=x, in_=xT_r[:, kt, ch * TOKC:(ch + 1) * TOKC])
                    x_tiles[(ch, kt)] = x

            # ---- filler machinery ----
            # pe_q: ordered (marker, thunk) list — QKV chunks, V transposes.
            # pr_q: (ready_iter, thunk) list — projection work of done groups.
            pe_q = []
            pr_q = []
            iter_ctr = [0]

            def pop_fillers(pr_budget=2, pe_budget=1):
                n = 0
                while pr_q and n < pr_budget and pr_q[0][0] <= iter_ctr[0]:
                    pr_q.pop(0)[1]()
                    n += 1
                n = 0
                while pe_q and n < pe_budget:
                    pe_q.pop(0)[1]()
                    n += 1

            def flush_until(marker):
                while pe_q and pe_q[0][0] <= marker:
                    pe_q.pop(0)[1]()

            def flush_all():
                while pe_q:
                    pe_q.pop(0)[1]()
                while pr_q:
                    pr_q.pop(0)[1]()

            # ---- building blocks ----
            def qkv_mm_group(ch, m):
                ps = ps_x.tile([128, TOKC], f32, tag="mm")
                for kt in range(KT_C):
                    nc.tensor.matmul(
                        ps[:, :],
                        w_t[kt][:, m * MQ:(m + 1) * MQ],
                        x_tiles[(ch, kt)][:, :],
                        start=(kt == 0), stop=(kt == KT_C - 1),
                    )
                dst = (qT_sb, kT_sb, vT_sb)[m]
                nc.vector.tensor_scalar_add(
                    out=dst[:, ch * TOKC:(ch + 1) * TOKC], in0=ps[:, :],
                    scalar1=b_sb[:, m:m + 1],
                )

            def vtrans_tile(b, kt):
                ps_t = ps_x.tile([128, 128], bf16, tag="mm")
                nc.tensor.transpose(
                    ps_t[:, :],
                    vT_sb[:, b * Tc + kt * 128:b * Tc + (kt + 1) * 128],
                    ident_bf[:, :],
                )
                for hh in range(hpc):
                    nc.vector.tensor_copy(
                        v_sb[b][:, kt, hh, 0:64],
                        ps_t[:, hh * HD:(hh + 1) * HD])

            def proj_mo(b, qc, yT_t, mo, on_act):
                q_sl = slice(b * Tc + qc * TOKC, b * Tc + (qc + 1) * TOKC)
                pso = ps_x.tile([128, TOKC], f32, tag="mm")
                nc.tensor.matmul(
                    pso[:, :],
                    wp_sb[:, mo * 128:(mo + 1) * 128],
                    yT_t[:, :],
                    start=True, stop=True,
                )
                o_t = opool.tile([128, TOKC], bf16, tag="o")
                if on_act:
                    nc.scalar.activation(out=o_t[:, :], in_=pso[:, :],
                                         func=AF.Copy)
                else:
                    nc.vector.tensor_copy(o_t[:, :], pso[:, :])
                nc.sync.dma_start(
                    out=outT[mo * 128:(mo + 1) * 128, q_sl], in_=o_t[:, :])

            def push_proj(b, qc, yT_t, delay=4):
                for mo in range(MO):
                    pr_q.append((
                        iter_ctr[0] + delay,
                        lambda mo=mo: proj_mo(b, qc, yT_t, mo,
                                              on_act=(mo % 4 == 3)),
                    ))

            DEPTH = 2

            def emit_group(b, qc):
                n_kt = (qc + 1) * DKT
                if qc == 0:
                    kts = list(range(DKT))
                else:
                    nd = list(range(0, qc * DKT))
                    dg = list(range(qc * DKT, qc * DKT + DKT))
                    kts = nd[:2] + dg + nd[2:]
                q_sl = slice(b * Tc + qc * TOKC, b * Tc + (qc + 1) * TOKC)
                yT_t = ypool.tile([128, TOKC], bf16, tag="yT")
                psy = [ps_y.tile([65, TOKC], f32, tag="y", name=f"psy{hh}")
                       for hh in range(hpc)]
                pend = []
                n_av = 0

                def emit_av(kt, e_t):
                    nonlocal n_av
                    for hh in range(hpc):
                        nc.tensor.matmul(
                            psy[hh][:, :],
                            v_sb[b][:, kt, hh, :],
                            e_t[:, hh, :],
                            start=(n_av == 0), stop=(n_av == n_kt - 1),
                        )
                    n_av += 1

                for kt in kts:
                    pss = ps_s.tile([128, hpc, TOKC], f32, tag="s")
                    for hh in range(hpc):
                        # disjoint PE row groups -> the two S matmuls
                        # overlap in the array
                        nc.tensor.matmul(
                            pss[:, hh, :],
                            kT_sb[hh * HD:(hh + 1) * HD,
                                  b * Tc + kt * 128:b * Tc + (kt + 1) * 128],
                            qT_sb[hh * HD:(hh + 1) * HD, q_sl],
                            start=True, stop=True,
                        )
                    e_t = epool.tile([128, hpc, TOKC], bf16, tag="e")
                    nc.scalar.activation(out=e_t[:, :, :], in_=pss[:, :, :],
                                         func=AF.Exp, scale=0.125)
                    di = kt - qc * DKT
                    if di >= 0:
                        # keep iff q >= k iff f - p - 128*di >= 0, same
                        # predicate for both head halves
                        nc.gpsimd.affine_select(
                            out=e_t[:, :, :], in_=e_t[:, :, :],
                            compare_op=mybir.AluOpType.is_ge,
                            fill=0.0,
                            base=-(di * 128),
                            pattern=[[0, hpc], [1, TOKC]],
                            channel_multiplier=-1,
                        )
                    pend.append((kt, e_t))
                    pop_fillers()
                    iter_ctr[0] += 1
                    if len(pend) > DEPTH:
                        emit_av(*pend.pop(0))
                for kt, e_t in pend:
                    emit_av(kt, e_t)

                for hh in range(hpc):
                    zrow = npool.tile([1, TOKC], f32, tag="z")
                    nc.vector.tensor_copy(zrow[:, :], psy[hh][64:65, :])
                    rc = npool.tile([1, TOKC], f32, tag="rc")
                    nc.vector.reciprocal_approx_fast(rc[:, :], zrow[:, :])
                    rcb = npool.tile([64, TOKC], f32, tag="rcb")
                    nc.gpsimd.partition_broadcast(rcb[:, :], rc[:, :])
                    nc.vector.tensor_mul(
                        yT_t[hh * HD:(hh + 1) * HD, :],
                        psy[hh][0:HD, :], rcb[:, :],
                    )
                return yT_t

            # ---- schedule ----
            for m in range(3):
                qkv_mm_group(0, m)
            for kt in range(DKT):
                vtrans_tile(0, kt)

            # filler inventory with ordering markers: marker value gates
            # correctness flushes before dependent attention groups
            def push_pe(marker, thunk):
                pe_q.append((marker, thunk))

            for ch in range(1, CH_PER_B + 2):        # ch 1..5
                for m in range(3):
                    push_pe(ch, lambda ch=ch, m=m: qkv_mm_group(ch, m))
                if ch < CH_PER_B:                    # b0 transposes kt 4..15
                    for kt in range(ch * DKT, (ch + 1) * DKT):
                        push_pe(ch, lambda kt=kt: vtrans_tile(0, kt))

            for qc in range(QC):                     # b0 attention
                flush_until(qc)                      # chunks/trans <= qc
                yT_t = emit_group(0, qc)
                push_proj(0, qc, yT_t)

            # remaining b1 inventory: trans for ch4/5, then ch6/7 + trans
            for kt in range(0, 2 * DKT):
                push_pe(10, lambda kt=kt: vtrans_tile(1, kt))
            for ch in range(CH_PER_B + 2, NCH):      # ch 6, 7
                for m in range(3):
                    push_pe(ch + 6, lambda ch=ch, m=m: qkv_mm_group(ch, m))
                for kt in range((ch - CH_PER_B) * DKT,
                                (ch - CH_PER_B + 1) * DKT):
                    push_pe(ch + 6, lambda kt=kt: vtrans_tile(1, kt))

            for qc in range(QC):                     # b1 attention
                flush_until(10 + qc)
                yT_t = emit_group(1, qc)
                push_proj(1, qc, yT_t)
            flush_all()

    nc.finalize()
    return nc


def prep_inputs(cfg, x, W_attn, b_attn, W_proj, b_proj):
    """Host-side sharding: returns per-core input dicts."""
    Bc, Tc, Cc, hpc = cfg["B"], cfg["T"], cfg["C"], cfg["HPC"]
    n_cores = (Cc // HD) // hpc
    BT = Bc * Tc
    MQ = hpc * HD

    x = np.ascontiguousarray(x, dtype=np.float32)
    xT = np.ascontiguousarray(x.reshape(BT, Cc).T)

    in_maps = []
    for c in range(n_cores):
        r0 = c * MQ
        rows = []
        for g in range(3):
            rows.append(np.arange(g * Cc + r0, g * Cc + r0 + MQ))
        rows = np.concatenate(rows)
        w_slice = W_attn[rows, :]                       # [384, C]
        wqkvT = np.ascontiguousarray(w_slice.T)         # [C, 384]
        bq = np.ascontiguousarray(b_attn[rows].reshape(MQ * 3, 1))
        import ml_dtypes
        wpT = np.ascontiguousarray(W_proj[:, r0:r0 + MQ].T).astype(ml_dtypes.bfloat16)
        in_maps.append({
            "xT": xT,
            "wqkvT": wqkvT.astype(np.float32),
            "bqkv": bq.astype(np.float32),
            "wpT": wpT,
        })
    return in_maps


def combine(cfg, results, b_proj):
    Bc, Tc, Cc = cfg["B"], cfg["T"], cfg["C"]
    acc = results[0]["outT"].astype(np.float32)
    for r in results[1:]:
        acc = acc + r["outT"].astype(np.float32)
    out = acc.T + b_proj[None, :]
    return np.ascontiguousarray(out.reshape(Bc, Tc, Cc).astype(np.float32))


_NC_CACHE = {}


def kernel(x, W_attn, b_attn, W_proj, b_proj):
    from concourse.bass_utils import run_bass_kernel_spmd

    cfg = _cfg_full()
    key = "full"
    if key not in _NC_CACHE:
        _NC_CACHE[key] = build_nc(cfg)
    nc = _NC_CACHE[key]
    in_maps = prep_inputs(cfg, np.asarray(x), np.asarray(W_attn),
                          np.asarray(b_attn), np.asarray(W_proj),
                          np.asarray(b_proj))
    res = run_bass_kernel_spmd(nc, in_maps, list(range(N_CORES)))
    return combine(cfg, res.results, np.asarray(b_proj, dtype=np.float32))
